# revision 19
# baseline (speedup 1.0000x reference)
"""BiAttention kernel for Trainium2, 8 NeuronCores, data-parallel over batch.

Reference computation (per batch b):
    S[i,j] = w1.c_i + w2.q_j + w3.(c_i*q_j)
    A      = softmax(S, axis=j)
    U[i]   = sum_j A[i,j] q_j
    bmax_i = max_j A[i,j]
    h      = sum_i bmax_i c_i
    G      = concat([c, U, c*U, c*h], axis=-1)

Key restructuring:
  - softmax over j is invariant to the s_c[i] term -> w1 is dead.
  - s_q[j] is added into S via an extra K=1 matmul (ones (x) s_q outer
    product accumulated into PSUM), so A = exp(S) directly; no row-max
    subtraction needed (|S| <= ~10 in this distribution, safe in fp32).
  - Z_i falls out of the U matmul via a ones-column appended to q.
  - bmax_i = (max_j A_raw) / Z_i.
  - matmuls run in float32r (full-rate PE mode, ~1 cycle/column vs 4 for
    fp32); rounding to fp32r happens at the PSUM->SBUF evacuation copies.
"""

import sys

if "/opt/trn_rl_repo" not in sys.path:
    sys.path.insert(0, "/opt/trn_rl_repo")

from contextlib import ExitStack

import numpy as np

import concourse.bass as bass
import concourse.bacc as bacc_mod
import concourse.tile as tile
from concourse import mybir
from concourse.bass_utils import run_bass_kernel_spmd
from concourse.masks import make_identity

B, Tc, Tq, D = 8, 4096, 1024, 256
P = 128
NT = Tc // P  # 32 context row-tiles
JC = Tq // P  # 8 question chunks
KC = D // P  # 2 feature chunks
N_CORES = 8
F32 = mybir.dt.float32
R32 = mybir.dt.float32r
BF16 = mybir.dt.bfloat16
EXP = mybir.ActivationFunctionType.Exp
import os
BF16A = bool(os.environ.get("BF16A"))  # bf16 A/A^T/U-matmul path
ADT = BF16 if BF16A else R32


def _build_program(repeat: int = 1) -> bass.Bass:
    nc = bacc_mod.Bacc()
    c_dram = nc.declare_dram_parameter("context", [Tc, D], F32, isOutput=False)
    q_dram = nc.declare_dram_parameter("question", [Tq, D], F32, isOutput=False)
    w_dram = nc.declare_dram_parameter("w", [3 * D, 1], F32, isOutput=False)
    g_dram = nc.declare_dram_parameter("out", [Tc, 4 * D], F32, isOutput=True)

    with ExitStack() as ctx:
        tc = ctx.enter_context(tile.TileContext(nc))
        singles = ctx.enter_context(tc.tile_pool(name="singles", bufs=1))
        work = ctx.enter_context(tc.tile_pool(name="work", bufs=3))
        dram = ctx.enter_context(tc.tile_pool(name="dram", bufs=1, space="DRAM"))
        ps_s = ctx.enter_context(tc.tile_pool(name="ps_s", bufs=3, space="PSUM"))
        ps_tp = ctx.enter_context(tc.tile_pool(name="ps_tp", bufs=3, space="PSUM"))
        ps_u = ctx.enter_context(tc.tile_pool(name="ps_u", bufs=2, space="PSUM"))

        # ---------------- prep (once per batch) ----------------
        ident = singles.tile([P, P], F32)
        nc.vector.memset(ident, 0.0)
        nc.vector.affine_select(
            out=ident,
            in_=ident,
            compare_op=mybir.AluOpType.not_equal,
            fill=1.0,
            base=0,
            pattern=[[-1, P]],
            channel_multiplier=1,
        )
        identr = singles.tile([P, P], ADT)
        nc.vector.tensor_copy(identr, ident)

        # w2 (fp32r, matmul lhsT) and w3 (fp32, used as a DVE scalar operand)
        wtmp = singles.tile([P, KC], F32)
        w3sc = singles.tile([P, KC], F32)
        for kc in range(KC):
            nc.sync.dma_start(
                out=wtmp[:, kc : kc + 1], in_=w_dram[D + kc * P : D + (kc + 1) * P, 0:1]
            )
            nc.sync.dma_start(
                out=w3sc[:, kc : kc + 1],
                in_=w_dram[2 * D + kc * P : 2 * D + (kc + 1) * P, 0:1],
            )
        w2sc = singles.tile([P, KC], R32)
        nc.vector.tensor_copy(w2sc, wtmp)

        # question: raw fp32 load, then a rounded fp32r copy with a ones column
        q_raw = singles.tile([P, JC, D], F32)
        nc.sync.dma_start(
            out=q_raw, in_=q_dram[:].rearrange("(jc p) d -> p jc d", p=P)
        )
        q_aug = singles.tile([P, JC, D + 4], ADT)
        if BF16A:
            nc.vector.memset(q_aug[:, :, D : D + 4], 0.0)
            nc.vector.memset(q_aug[:, :, D : D + 1], 1.0)
        else:
            nc.vector.memset(q_aug[:, :, D : D + 4].bitcast(F32), 0.0)
            nc.vector.memset(q_aug[:, :, D : D + 1].bitcast(F32), 1.0)
        nc.vector.tensor_copy(q_aug[:, :, 0:D], q_raw)

        # q^T via PE transposes; each psum tile evacuated twice:
        # once w3-scaled (S matmul rhs), once unscaled (s_q matvec rhs)
        qTu = [singles.tile([P, Tq], R32, name=f"qTu{k}") for k in range(KC)]
        qTw = [singles.tile([P, Tq], R32, name=f"qTw{k}") for k in range(KC)]
        for kc in range(KC):
            for jg in range(2):
                tp = ps_tp.tile([P, 512], F32, tag="tp")
                for j4 in range(4):
                    jc = jg * 4 + j4
                    nc.tensor.transpose(
                        tp[:, j4 * P : (j4 + 1) * P],
                        q_raw[:, jc, kc * P : (kc + 1) * P],
                        ident,
                    )
                nc.vector.tensor_copy(qTu[kc][:, jg * 512 : (jg + 1) * 512], tp)
                nc.vector.tensor_scalar_mul(
                    qTw[kc][:, jg * 512 : (jg + 1) * 512], tp, w3sc[:, kc : kc + 1]
                )

        # s_q = q @ w2 in natural [1, Tq] layout; ones row for the K=1 add-matmul
        ones_row = singles.tile([1, P], R32)
        nc.vector.memset(ones_row.bitcast(F32), 1.0)
        sq_sb = singles.tile([1, Tq], R32)
        for nb in range(2):
            sq_ps = ps_u.tile([1, 512], F32, tag="u")
            for kc in range(KC):
                nc.tensor.matmul(
                    sq_ps,
                    lhsT=w2sc[:, kc : kc + 1],
                    rhs=qTu[kc][:, nb * 512 : (nb + 1) * 512],
                    start=(kc == 0),
                    stop=(kc == KC - 1),
                )
            nc.vector.tensor_copy(sq_sb[:, nb * 512 : (nb + 1) * 512], sq_ps)

        c_all = singles.tile([P, NT, D], F32)
        c_r = singles.tile([P, NT * D], R32)
        b_all = singles.tile([P, NT], R32)

        # -------- main loop (+ epilogue), optionally HW-looped for timing --------
        args = (nc, tc, work, ps_s, ps_tp, ps_u, singles, dram, c_dram, g_dram,
                ident, identr, q_aug, qTw, ones_row, sq_sb, c_all, c_r, b_all)
        if repeat == 1:
            _main_loop(*args)
        else:
            hint = (mybir.EngineType.PE, mybir.EngineType.Activation,
                    mybir.EngineType.DVE, mybir.EngineType.SP,
                    mybir.EngineType.Pool)
            with tc.For_i(0, repeat, 1, hint_engines=hint):
                _main_loop(*args)

    nc.finalize()
    return nc


def _main_loop(nc, tc, work, ps_s, ps_tp, ps_u, singles, dram, c_dram, g_dram,
               ident, identr, q_aug, qTw, ones_row, sq_sb, c_all, c_r, b_all):
    for t in range(NT):
        c_t = c_all[:, t, :]
        if t % 4 == 0:
            nc.sync.dma_start(
                out=c_all[:, t : t + 4, :],
                in_=c_dram[t * P : (t + 4) * P, :].rearrange(
                    "(g p) d -> p g d", p=P
                ),
            )

        # c^T for this row-tile (2 transpose blocks -> one psum tile -> one
        # rounding evac on DVE)
        tp = ps_tp.tile([P, 512], F32, tag="tp")
        for kc in range(KC):
            nc.tensor.transpose(
                tp[:, kc * P : (kc + 1) * P], c_t[:, kc * P : (kc + 1) * P], ident
            )
        cT = work.tile([P, KC * P], R32, tag="ct")
        nc.vector.tensor_copy(cT, tp[:, 0 : KC * P])

        # S = (c*w3) @ q^T + 1 (x) s_q  (K=1 matmul adds the s_q row),
        # then A = exp(S) in one ACT op
        A_sb = work.tile([P, Tq], ADT, tag="A")
        for nb in range(2):
            sl = slice(nb * 512, (nb + 1) * 512)
            s_ps = ps_s.tile([P, 512], F32, tag="s")
            for kc in range(KC):
                nc.tensor.matmul(
                    s_ps,
                    lhsT=cT[:, kc * P : (kc + 1) * P],
                    rhs=qTw[kc][:, sl],
                    start=(kc == 0),
                    stop=False,
                )
            nc.tensor.matmul(
                s_ps, lhsT=ones_row, rhs=sq_sb[:, sl], start=False, stop=True
            )
            nc.scalar.activation(A_sb[:, sl], s_ps, EXP)

        # bZ = max_j A_raw
        bZ = work.tile([P, 1], F32, tag="bz")
        nc.vector.tensor_reduce(
            out=bZ,
            in_=A_sb if BF16A else A_sb.bitcast(F32),
            axis=mybir.AxisListType.X,
            op=mybir.AluOpType.max,
        )

        # A^T via PE transposes (8 blocks, 2 psum tiles, 2 rounding evacs on ACT)
        AT = work.tile([P, JC, P], ADT, tag="AT")
        for jg in range(2):
            tp2 = ps_tp.tile([P, 512], ADT, tag="tp")
            for j4 in range(4):
                jc = jg * 4 + j4
                nc.tensor.transpose(
                    tp2[:, j4 * P : (j4 + 1) * P],
                    A_sb[:, jc * P : (jc + 1) * P],
                    identr,
                )
            if BF16A and jg == 1:
                nc.vector.tensor_copy(AT[:, jg * 4 : (jg + 1) * 4, :], tp2)
            else:
                nc.scalar.copy(AT[:, jg * 4 : (jg + 1) * 4, :], tp2)

        # Utilde = A_raw @ [q | 1] -> cols 0..255 = U*Z, col 256 = Z
        u_ps = ps_u.tile([P, D + 4], F32, tag="u")
        for jc in range(JC):
            nc.tensor.matmul(
                u_ps,
                lhsT=AT[:, jc, :],
                rhs=q_aug[:, jc, 0 : D + 4],
                start=(jc == 0),
                stop=(jc == JC - 1),
            )

        rZ = work.tile([P, 1], F32, tag="rz")
        nc.vector.reciprocal(rZ, u_ps[:, D : D + 1])
        U_sb = work.tile([P, D], F32, tag="U")
        nc.vector.tensor_scalar_mul(U_sb, u_ps[:, 0:D], rZ)
        nc.vector.tensor_scalar_mul(b_all[:, t : t + 1], bZ, rZ)

        nc.vector.tensor_copy(c_r[:, t * D : (t + 1) * D], c_t)

        cU = work.tile([P, D], F32, tag="cU")
        nc.gpsimd.tensor_mul(cU, c_t, U_sb)

        g_rows = g_dram[t * P : (t + 1) * P, :]
        nc.gpsimd.dma_start(out=g_rows[:, 0:D], in_=c_t)
        nc.gpsimd.dma_start(out=g_rows[:, D : 2 * D], in_=U_sb)
        nc.gpsimd.dma_start(out=g_rows[:, 2 * D : 3 * D], in_=cU)

    # ---------------- epilogue: h = sum_i bmax_i c_i, then c*h ----------------
    h_ps = ps_u.tile([1, D], F32, tag="u")
    for t in range(NT):
        nc.tensor.matmul(
            h_ps,
            lhsT=b_all[:, t : t + 1],
            rhs=c_r[:, t * D : (t + 1) * D],
            start=(t == 0),
            stop=(t == NT - 1),
        )
    h_sb = work.tile([1, D], F32, tag="hsb")
    nc.vector.tensor_copy(h_sb, h_ps)
    h_dram = dram.tile([1, D], F32)
    nc.sync.dma_start(out=h_dram, in_=h_sb)
    h_bcast = work.tile([P, D], F32, tag="hbc")
    nc.sync.dma_start(out=h_bcast, in_=h_dram.to_broadcast([P, D]))

    for tg in range(NT // 4):
        ch4 = work.tile([P, 4, D], F32, tag="ch4")
        for i in range(4):
            t = tg * 4 + i
            if i == 0:
                nc.gpsimd.tensor_mul(ch4[:, i, :], c_all[:, t, :], h_bcast)
            else:
                nc.vector.tensor_mul(ch4[:, i, :], c_all[:, t, :], h_bcast)
        nc.sync.dma_start(
            out=g_dram[tg * 4 * P : (tg + 1) * 4 * P, 3 * D : 4 * D].rearrange(
                "(g p) d -> p g d", p=P
            ),
            in_=ch4,
        )


_NC_CACHE = None


def kernel(context, question, w):
    global _NC_CACHE
    context = np.asarray(context, dtype=np.float32)
    question = np.asarray(question, dtype=np.float32)
    w = np.asarray(w, dtype=np.float32)

    if _NC_CACHE is None:
        _NC_CACHE = _build_program()
    nc = _NC_CACHE

    in_maps = [
        {"context": context[b], "question": question[b], "w": w} for b in range(B)
    ]
    res = run_bass_kernel_spmd(nc, in_maps, list(range(N_CORES)))
    return np.stack([res.results[b]["out"] for b in range(B)], axis=0)
